# revision 8
# baseline (speedup 1.0000x reference)
import sys

sys.path.insert(0, "/opt/trn_rl_repo")

import numpy as np

import concourse.bacc as bacc
import concourse.bass as bass
import concourse.mybir as mybir
from concourse.bass_utils import run_bass_kernel_spmd
from concourse.tile import TileContext

# Problem constants (hardcoded per contract)
B, NPRE, NPOST = 32, 1000, 1000
DT = 0.1
TAU_AMPA = 2.0
TAU_NMDA = 100.0
NMDA_FRAC = 0.3

N_CORES = 8
BL = B // N_CORES          # local batch per core = 4
P = 125                    # partitions per pre-chunk
NCH = NPRE // P            # 8 chunks

D_A = float(np.exp(np.float32(-DT / TAU_AMPA)))
D_N = float(np.exp(np.float32(-DT / TAU_NMDA)))
K_A = (1.0 - D_A) * (1.0 - NMDA_FRAC)
K_N = (1.0 - D_N) * NMDA_FRAC

_CACHE = {}


def _build_program(repeat=1, group=1, mode="full", io_bufs=None):
    """group: pre-chunks per DMA/compute tile. mode: 'full' or 'loads_diag'
    (diagnostic: no big stores, light PE, times the read stream)."""
    assert NCH % group == 0
    NG = NCH // group
    if io_bufs is None:
        io_bufs = {1: 5, 2: 3, 4: 2}[group]
    nc = bacc.Bacc("TRN2", target_bir_lowering=False, debug=False)
    f32 = mybir.dt.float32

    ampa = nc.dram_tensor("ampa", [BL, NPRE, NPOST], f32, kind="ExternalInput")
    nmda = nc.dram_tensor("nmda", [BL, NPRE, NPOST], f32, kind="ExternalInput")
    # gc = g_max * connectivity, host-pretransposed to [P, NCH*NPOST]
    gc = nc.dram_tensor("gc", [P, NCH * NPOST], f32, kind="ExternalInput")
    # coef[p, w*BL*NCH + b*NCH + c] for w in {ampa,nmda}
    coef = nc.dram_tensor("coef", [P, 2 * BL * NCH], f32, kind="ExternalInput")

    ampa_o = nc.dram_tensor("ampa_o", [BL, NPRE, NPOST], f32, kind="ExternalOutput")
    nmda_o = nc.dram_tensor("nmda_o", [BL, NPRE, NPOST], f32, kind="ExternalOutput")
    tot_a = nc.dram_tensor("tot_a", [BL, NPOST], f32, kind="ExternalOutput")
    tot_n = nc.dram_tensor("tot_n", [BL, NPOST], f32, kind="ExternalOutput")

    NH = NPOST // 2  # matmul N tile (<=512)

    def state_ap(t, b, g):
        sl = t[b, g * group * P : (g + 1) * group * P, :]
        if group == 1:
            return sl
        return sl.rearrange("(c p) f -> p c f", p=P)

    with TileContext(nc) as tc:
        for _rep in range(repeat):
            with (
                tc.tile_pool(name="const", bufs=1) as cpool,
                tc.tile_pool(name="io", bufs=io_bufs) as io,
                tc.tile_pool(name="tot", bufs=2) as tpool,
                tc.tile_pool(name="psum", bufs=2, space=bass.MemorySpace.PSUM) as pp,
            ):
                gc_sb = cpool.tile([P, NCH * NPOST], f32, tag="gc")
                coef_sb = cpool.tile([P, 2 * BL * NCH], f32, tag="coef")
                ones_sb = cpool.tile([P, 1], f32, tag="ones")

                nc.gpsimd.memset(ones_sb[:], 1.0)
                nc.sync.dma_start(out=coef_sb[:], in_=coef[:])
                for c in range(NCH):
                    nc.sync.dma_start(
                        out=gc_sb[:, c * NPOST : (c + 1) * NPOST],
                        in_=gc[:, c * NPOST : (c + 1) * NPOST],
                    )

                for b in range(BL):
                    ps = {
                        (w, h): pp.tile([1, NH], f32, name=f"ps_{w}{h}", tag=f"ps_{w}{h}")
                        for w in range(2)
                        for h in range(2)
                    }
                    for g in range(NG):
                        shape = [P, NPOST] if group == 1 else [P, group, NPOST]
                        a_t = io.tile(shape, f32, tag="a_t")
                        nc.sync.dma_start(out=a_t[:], in_=state_ap(ampa, b, g))
                        if mode != "loads_half":
                            n_t = io.tile(shape, f32, tag="n_t")
                            nc.sync.dma_start(out=n_t[:], in_=state_ap(nmda, b, g))

                        if mode in ("loads_diag", "loads_half"):
                            for cc in range(group):
                                c = g * group + cc
                                at_c = a_t[:] if group == 1 else a_t[:, cc, :]
                                nc.tensor.matmul(
                                    ps[(0, 0)][:], ones_sb[:], at_c[:, 0:NH],
                                    start=(c == 0), stop=(c == NCH - 1),
                                )
                                if mode == "loads_half":
                                    continue
                                nt_c = n_t[:] if group == 1 else n_t[:, cc, :]
                                nc.tensor.matmul(
                                    ps[(1, 0)][:], ones_sb[:], nt_c[:, 0:NH],
                                    start=(c == 0), stop=(c == NCH - 1),
                                )
                            continue

                        sga = io.tile(shape, f32, tag="sga")
                        sgn = io.tile(shape, f32, tag="sgn")
                        for cc in range(group):
                            c = g * group + cc
                            gslc = gc_sb[:, c * NPOST : (c + 1) * NPOST]
                            sga_c = sga[:] if group == 1 else sga[:, cc, :]
                            sgn_c = sgn[:] if group == 1 else sgn[:, cc, :]
                            ia = b * NCH + c
                            inn = BL * NCH + b * NCH + c
                            nc.scalar.activation(
                                sga_c, gslc, mybir.ActivationFunctionType.Copy,
                                scale=coef_sb[:, ia : ia + 1],
                            )
                            nc.scalar.activation(
                                sgn_c, gslc, mybir.ActivationFunctionType.Copy,
                                scale=coef_sb[:, inn : inn + 1],
                            )

                        ao = io.tile(shape, f32, tag="ao")
                        no = io.tile(shape, f32, tag="no")
                        nc.vector.scalar_tensor_tensor(
                            out=ao[:], in0=a_t[:], scalar=D_A, in1=sga[:],
                            op0=mybir.AluOpType.mult, op1=mybir.AluOpType.add,
                        )
                        nc.vector.scalar_tensor_tensor(
                            out=no[:], in0=n_t[:], scalar=D_N, in1=sgn[:],
                            op0=mybir.AluOpType.mult, op1=mybir.AluOpType.add,
                        )

                        nc.gpsimd.dma_start(out=state_ap(ampa_o, b, g), in_=ao[:])
                        nc.gpsimd.dma_start(out=state_ap(nmda_o, b, g), in_=no[:])

                        for cc in range(group):
                            c = g * group + cc
                            ao_c = ao[:] if group == 1 else ao[:, cc, :]
                            no_c = no[:] if group == 1 else no[:, cc, :]
                            for h in range(2):
                                nc.tensor.matmul(
                                    ps[(0, h)][:], ones_sb[:], ao_c[:, h * NH : (h + 1) * NH],
                                    start=(c == 0), stop=(c == NCH - 1),
                                )
                                nc.tensor.matmul(
                                    ps[(1, h)][:], ones_sb[:], no_c[:, h * NH : (h + 1) * NH],
                                    start=(c == 0), stop=(c == NCH - 1),
                                )

                    ta = tpool.tile([1, NPOST], f32, tag="ta")
                    tn = tpool.tile([1, NPOST], f32, tag="tn")
                    if mode in ("loads_diag", "loads_half"):
                        nc.scalar.copy(ta[:, 0:NH], ps[(0, 0)][:])
                        if mode == "loads_half":
                            nc.scalar.copy(tn[:, 0:NH], ps[(0, 0)][:])
                        else:
                            nc.scalar.copy(tn[:, 0:NH], ps[(1, 0)][:])
                    else:
                        for h in range(2):
                            nc.scalar.copy(ta[:, h * NH : (h + 1) * NH], ps[(0, h)][:])
                            nc.scalar.copy(tn[:, h * NH : (h + 1) * NH], ps[(1, h)][:])
                    nc.gpsimd.dma_start(out=tot_a[b : b + 1, :], in_=ta[:])
                    nc.gpsimd.dma_start(out=tot_n[b : b + 1, :], in_=tn[:])

    nc.compile()
    return nc


def _get_program():
    if "nc" not in _CACHE:
        _CACHE["nc"] = _build_program(group=2)
    return _CACHE["nc"]


def _prep_shared(g_max, connectivity, pre_rates, modulation):
    gc = (g_max * connectivity).astype(np.float32)  # [NPRE, NPOST]
    # pre-transpose: [NPRE, NPOST] -> [P, NCH*NPOST] with row p holding chunks c
    gc_t = np.ascontiguousarray(
        gc.reshape(NCH, P, NPOST).transpose(1, 0, 2).reshape(P, NCH * NPOST)
    )
    rm = pre_rates * modulation[:, None]  # [B, NPRE]
    ca = (rm * K_A).astype(np.float32)
    cn = (rm * K_N).astype(np.float32)

    def pack(x):  # [BL, NPRE] -> [P, BL*NCH]
        return np.ascontiguousarray(
            x.reshape(BL, NCH, P).transpose(2, 0, 1).reshape(P, BL * NCH)
        )

    coefs = []
    for i in range(N_CORES):
        sl = slice(i * BL, (i + 1) * BL)
        coefs.append(
            np.concatenate([pack(ca[sl]), pack(cn[sl])], axis=1)  # [P, 2*BL*NCH]
        )
    return gc_t, coefs


def kernel(pre_rates, modulation, g_max, connectivity, ampa_state, nmda_state):
    pre_rates = np.asarray(pre_rates, dtype=np.float32)
    modulation = np.asarray(modulation, dtype=np.float32)
    g_max = np.asarray(g_max, dtype=np.float32)
    connectivity = np.asarray(connectivity, dtype=np.float32)
    ampa_state = np.asarray(ampa_state, dtype=np.float32)
    nmda_state = np.asarray(nmda_state, dtype=np.float32)

    nc = _get_program()
    gc_t, coefs = _prep_shared(g_max, connectivity, pre_rates, modulation)

    in_maps = []
    for i in range(N_CORES):
        sl = slice(i * BL, (i + 1) * BL)
        in_maps.append(
            {
                "ampa": ampa_state[sl],
                "nmda": nmda_state[sl],
                "gc": gc_t,
                "coef": coefs[i],
            }
        )

    res = run_bass_kernel_spmd(nc, in_maps, list(range(N_CORES)))
    r = res.results
    ampa_new = np.concatenate([r[i]["ampa_o"] for i in range(N_CORES)], axis=0)
    nmda_new = np.concatenate([r[i]["nmda_o"] for i in range(N_CORES)], axis=0)
    total_ampa = np.concatenate([r[i]["tot_a"] for i in range(N_CORES)], axis=0)
    total_nmda = np.concatenate([r[i]["tot_n"] for i in range(N_CORES)], axis=0)
    return ampa_new, nmda_new, total_ampa, total_nmda
